# revision 30
# baseline (speedup 1.0000x reference)
"""AdaptiveFocalLoss on 8 TRN2 NeuronCores (Bass/Tile).

Data-parallel over batch N (8 images -> 8 cores). Per-core shard:
logits (16, 512*512) f32, target (512*512,) int.

Per-core device computation (positions P = 262144, C = 16):
  layout: SBUF [128, F] with partition p = 16*g + c   (g = spatial group)
  expX   = exp(logits)                     (ACT, bf16 out)
  M      = (T_rep == c_partition) * expX   (DVE scalar_tensor_tensor)
  D      = sum_c expX          via PE matmul with block-diag Sel8  [8,F]
  e_t    = sum_c M             via PE matmul with Sel8
  e''    = sum_c alpha_c * M   via PE matmul with alpha-weighted Sel8
  (16 matmuls at staggered PSUM partition offsets build [128,512] tiles)
  lp     = log e_t - log D     (= log p_true)
  a      = exp(log e'' - log e_t)  (= alpha[target])
  focal  = a * (1 - p)^2 * (-lp)
  loss   = sum(focal)
Class counts: 16x tensor_scalar(is_equal, accum) histogram + AllReduce
across the 8 cores; alpha computed on-device from global counts.
Host: sums the per-core partial sums, divides by (numel + eps).
"""

import sys

sys.path.insert(0, "/opt/trn_rl_repo")

import numpy as np
import ml_dtypes

import concourse.bass as bass
import concourse.bacc as bacc
import concourse.tile as tile
from concourse import mybir
from concourse.bass_utils import run_bass_kernel_spmd

# ---- problem constants (hardcoded; kernel.py must be self-contained) ----
N, C, H, W = 8, 16, 512, 512
POS = H * W          # positions per core = 262144
G = 8                # spatial groups -> partition = 16*g + c
FTOT = POS // G      # free columns in (g,c) layout = 32768
CHUNK = 2048         # sweep chunk columns
NCHUNK = FTOT // CHUNK          # 16
SUB = 512            # matmul moving free dim (one PSUM bank)
SUBS_PER_SC = 16     # matmuls per super-chunk -> fills 128 partitions
SC_COLS = SUB * SUBS_PER_SC     # 8192 columns per super-chunk
NSC = FTOT // SC_COLS           # 4 super-chunks
CHUNKS_PER_SC = SC_COLS // CHUNK  # 4

GAMMA = 2.0
SMOOTH = 1e-8
ALPHA_SMOOTH = 0.1

FP32 = mybir.dt.float32
BF16 = mybir.dt.bfloat16
AX = mybir.AxisListType
OP = mybir.AluOpType
AF = mybir.ActivationFunctionType


def build_nc(compile_graph=True, use_collective=True, use_late=True):
    nc = bacc.Bacc("TRN2", target_bir_lowering=False, debug=False,
                   num_devices=8)

    x_ext = nc.declare_dram_parameter("x", [C, POS], FP32, isOutput=False)
    tpos_ext = nc.declare_dram_parameter("tpos", [128, POS // 128], BF16,
                                         isOutput=False)
    sel8_ext = nc.declare_dram_parameter("sel8", [128, G], BF16, isOutput=False)
    ones_ext = nc.declare_dram_parameter("ones128", [128, 1], FP32,
                                         isOutput=False)
    ccol_ext = nc.declare_dram_parameter("ccol", [128, 1], FP32, isOutput=False)
    out_ext = nc.declare_dram_parameter("out", [128, NSC], FP32, isOutput=True)
    dbg_ext = nc.declare_dram_parameter("dbg", [4, C], FP32, isOutput=True)

    # (g,c)-layout view of logits: partition p=(g,c) -> dram c*POS + g*FTOT + f
    x_gc = bass.AP(
        tensor=x_ext,
        offset=0,
        ap=[[FTOT, G], [POS, C], [1, FTOT]],
    )

    with tile.TileContext(nc) as tc:
        with (
            tc.tile_pool(name="singles", bufs=1) as singles,
            tc.tile_pool(name="xp", bufs=3) as xp,
            tc.tile_pool(name="exp", bufs=3) as exp_pool,
            tc.tile_pool(name="trepp", bufs=3) as trepp,
            tc.tile_pool(name="pos", bufs=2) as pos_pool,
            tc.tile_pool(name="late", bufs=NSC) as late_pool,
            tc.tile_pool(name="tiny", bufs=2) as tiny,
            tc.tile_pool(name="psA", bufs=2, space="PSUM") as psA,
            tc.tile_pool(name="psB", bufs=2, space="PSUM") as psB,
            tc.tile_pool(name="dram", bufs=1, space="DRAM") as dram,
        ):
            # ---------------- constants / small inputs ----------------
            # DMA-landed constants are re-copied by the vector engine: PE's
            # LOAD_WEIGHTS and DVE's STT have a single sync-wait slot, so
            # their dependencies must all ride the one DVE semaphore.
            sel8_in = singles.tile([128, G], BF16)
            nc.sync.dma_start(out=sel8_in, in_=sel8_ext[:, :])
            sel8 = singles.tile([128, G], BF16)
            nc.vector.tensor_copy(out=sel8, in_=sel8_in)
            ones_in = singles.tile([128, 1], FP32)
            nc.sync.dma_start(out=ones_in, in_=ones_ext[:, :])
            ones128 = singles.tile([128, 1], FP32)
            nc.vector.tensor_copy(out=ones128, in_=ones_in)
            ccol_in = singles.tile([128, 1], FP32)
            nc.sync.dma_start(out=ccol_in, in_=ccol_ext[:, :])
            ccol = singles.tile([128, 1], FP32)
            nc.vector.tensor_copy(out=ccol, in_=ccol_in)
            tpos = singles.tile([128, POS // 128], BF16)
            nc.sync.dma_start(out=tpos, in_=tpos_ext[:, :])

            # ---------------- histogram -> counts ----------------
            cnt = singles.tile([128, C], FP32)
            for c in range(C):
                scr = tiny.tile([128, POS // 128], BF16, tag="hscr")
                nc.vector.tensor_scalar(
                    out=scr, in0=tpos, scalar1=float(c), scalar2=0.0,
                    op0=OP.is_equal, op1=OP.add, accum_out=cnt[:, c:c + 1],
                )
            cnt_ps = psA.tile([1, C], FP32, tag="cntps")
            nc.tensor.matmul(cnt_ps, lhsT=ones128, rhs=cnt, start=True,
                             stop=True)
            cnt_sb = singles.tile([1, C], FP32)
            nc.vector.tensor_copy(out=cnt_sb, in_=cnt_ps)

            cnt_g = singles.tile([1, C], FP32)
            if use_collective:
                cc_in = dram.tile([1, C], FP32)
                cc_out = dram.tile([1, C], FP32)
                nc.gpsimd.dma_start(out=cc_in[:], in_=cnt_sb)
                nc.gpsimd.collective_compute(
                    "AllReduce", OP.add,
                    replica_groups=[list(range(8))],
                    ins=[cc_in.opt()], outs=[cc_out.opt()],
                )
                nc.gpsimd.dma_start(out=cnt_g, in_=cc_out[:])
            else:
                # debug path: local counts scaled by 8 (uniform-ish targets)
                nc.vector.tensor_scalar_mul(cnt_g, cnt_sb, 8.0)

            # ---------------- alpha from global counts ----------------
            # freq = cnt/total ; w = 1/(freq + 0.1); present = cnt > 0
            # alpha = present ? w/sum(present*w) : 1.0
            wv = singles.tile([1, C], FP32)
            nc.vector.tensor_scalar(
                out=wv, in0=cnt_g, scalar1=1.0 / float(N * POS),
                scalar2=ALPHA_SMOOTH, op0=OP.mult, op1=OP.add,
            )
            nc.vector.reciprocal(out=wv, in_=wv)
            pres = singles.tile([1, C], FP32)
            nc.vector.tensor_scalar(
                out=pres, in0=cnt_g, scalar1=0.0, scalar2=None, op0=OP.is_gt,
            )
            wp = singles.tile([1, C], FP32)
            nc.vector.tensor_mul(wp, wv, pres)
            wsum = singles.tile([1, 1], FP32)
            nc.vector.tensor_reduce(out=wsum, in_=wp, axis=AX.X, op=OP.add)
            nc.vector.reciprocal(out=wsum, in_=wsum)
            alpha = singles.tile([1, C], FP32)
            # alpha = wp * (1/wsum) + (1 - pres)
            nc.vector.tensor_scalar(
                out=alpha, in0=wp, scalar1=wsum, scalar2=None, op0=OP.mult,
            )
            omp = singles.tile([1, C], FP32)
            nc.vector.tensor_scalar(
                out=omp, in0=pres, scalar1=-1.0, scalar2=1.0,
                op0=OP.mult, op1=OP.add,
            )
            nc.vector.tensor_add(alpha, alpha, omp)

            nc.gpsimd.dma_start(out=dbg_ext[0:1, :], in_=cnt_g)
            nc.gpsimd.dma_start(out=dbg_ext[1:2, :], in_=alpha)

            # alpha -> [128,1] column (alpha_col[p] = alpha[p % 16])
            al_dram = dram.tile([1, C], FP32)
            nc.gpsimd.dma_start(out=al_dram[:], in_=alpha)
            alpha_in = singles.tile([128, 1], FP32)
            al_bcast = bass.AP(
                tensor=al_dram.tensor,
                offset=al_dram.offset,
                ap=[[0, G], [1, C], [1, 1]],
            )
            nc.gpsimd.dma_start(out=alpha_in, in_=al_bcast)
            alpha_col = singles.tile([128, 1], FP32)
            nc.vector.tensor_copy(out=alpha_col, in_=alpha_in)
            sel8a = singles.tile([128, G], BF16)
            nc.vector.tensor_scalar(
                out=sel8a, in0=sel8, scalar1=alpha_col, scalar2=None,
                op0=OP.mult,
            )

            # ---------------- main sweep ----------------
            m_all = singles.tile([128, FTOT], BF16)
            loss_col = singles.tile([128, NSC], FP32)
            d_tiles = {}
            e_tiles = {}
            le_tiles = {}
            f1_tiles = {}

            for k in range(NCHUNK):
                col0 = k * CHUNK
                cols = slice(col0, col0 + CHUNK)

                x_t = xp.tile([128, CHUNK], FP32, tag="x")
                nc.sync.dma_start(out=x_t, in_=x_gc[:, :, cols])

                # T_rep chunk: tpos partition (16g + k) -> dst partitions
                # (16g + c) for all c  [src partition stride 16, c step 0]
                # tpos partition (8k + g) holds t[g*FTOT + k*CHUNK : +CHUNK];
                # broadcast each row to the 16 channel partitions of group g.
                trep = trepp.tile([128, CHUNK], BF16, tag="trep")
                t8_bcast = tpos[G * k:G * k + G, None, :].to_broadcast(
                    (G, C, CHUNK))
                nc.gpsimd.dma_start(out=trep, in_=t8_bcast)

                ex = exp_pool.tile([128, CHUNK], BF16, tag="ex")
                nc.scalar.activation(out=ex, in_=x_t, func=AF.Exp)

                # STT's 64B encoding has one sync-wait slot; absorb the
                # trep-DMA wait into a cheap DVE op so the stt only waits
                # on the ACT semaphore.
                touch = tiny.tile([1, 1], BF16, tag="touch")
                nc.vector.tensor_copy(out=touch, in_=trep[0:1, 0:1])
                nc.vector.scalar_tensor_tensor(
                    out=m_all[:, cols], in0=trep, scalar=ccol, in1=ex,
                    op0=OP.is_equal, op1=OP.mult,
                )

                s, j0 = divmod(k, CHUNKS_PER_SC)
                if j0 == 0:
                    d_tile = psA.tile([128, SUB], FP32, tag="D")
                    e_tile = psB.tile([128, SUB], FP32, tag="E")
                    d_tiles[s] = d_tile
                    e_tiles[s] = e_tile
                # data-as-weights: lhsT = 128-col data block, rhs = Sel8.
                # out[f, g] = sum_c block[(g,c), f] -> [128 positions, 8]
                for j in range(CHUNK // 128):
                    bb = j0 * (CHUNK // 128) + j  # block within super-chunk
                    blk = slice(col0 + j * 128, col0 + (j + 1) * 128)
                    nc.tensor.matmul(
                        d_tiles[s][:, 8 * bb:8 * bb + 8],
                        lhsT=ex[:, j * 128:(j + 1) * 128], rhs=sel8,
                        start=True, stop=True,
                    )
                    nc.tensor.matmul(
                        e_tiles[s][:, 8 * bb:8 * bb + 8],
                        lhsT=m_all[:, blk], rhs=sel8,
                        start=True, stop=True,
                    )

                if j0 == CHUNKS_PER_SC - 1:
                    # early epilogue for super-chunk s
                    lD = pos_pool.tile([128, SUB], FP32, tag="lD")
                    nc.scalar.activation(out=lD, in_=d_tiles[s], func=AF.Ln)
                    le = late_pool.tile([128, SUB], FP32, tag="le")
                    nc.scalar.activation(out=le, in_=e_tiles[s], func=AF.Ln)
                    le_tiles[s] = le
                    lp = pos_pool.tile([128, SUB], FP32, tag="lp")
                    nc.vector.tensor_sub(lp, le, lD)
                    p = pos_pool.tile([128, SUB], FP32, tag="p")
                    nc.scalar.activation(out=p, in_=lp, func=AF.Exp)
                    u_t = pos_pool.tile([128, SUB], FP32, tag="u")
                    nc.scalar.activation(out=u_t, in_=p, func=AF.Copy,
                                         bias=1.0, scale=-1.0)
                    w_t = pos_pool.tile([128, SUB], FP32, tag="w")
                    nc.scalar.activation(out=w_t, in_=u_t, func=AF.Square)
                    f1 = late_pool.tile([128, SUB], FP32, tag="f1")
                    nc.vector.tensor_mul(f1, w_t, lp)
                    f1_tiles[s] = f1

            # ---------------- alpha-weighted pass + late epilogue ----------
            for s in range(NSC if use_late else 0):
                epp = psB.tile([128, SUB], FP32, tag="EPP")
                for bb in range(SC_COLS // 128):
                    col0 = s * SC_COLS + bb * 128
                    nc.tensor.matmul(
                        epp[:, 8 * bb:8 * bb + 8],
                        lhsT=m_all[:, col0:col0 + 128], rhs=sel8a,
                        start=True, stop=True,
                    )
                lepp = pos_pool.tile([128, SUB], FP32, tag="lepp")
                nc.scalar.activation(out=lepp, in_=epp, func=AF.Ln)
                la = pos_pool.tile([128, SUB], FP32, tag="la")
                nc.vector.tensor_sub(la, lepp, le_tiles[s])
                a_t = pos_pool.tile([128, SUB], FP32, tag="a")
                nc.scalar.activation(out=a_t, in_=la, func=AF.Exp)
                f2 = pos_pool.tile([128, SUB], FP32, tag="f2")
                nc.vector.tensor_mul(f2, f1_tiles[s], a_t)
                nc.vector.tensor_reduce(
                    out=loss_col[:, s:s + 1], in_=f2, axis=AX.X, op=OP.add)

            if not use_late:
                # debug path: unweighted focal sum
                for s in range(NSC):
                    nc.vector.tensor_reduce(
                        out=loss_col[:, s:s + 1], in_=f1_tiles[s],
                        axis=AX.X, op=OP.add)

            nc.sync.dma_start(out=out_ext[:, :], in_=loss_col)

    if compile_graph:
        nc.compile()
    return nc


_CACHED = {}


def _get_nc():
    if "nc" not in _CACHED:
        _CACHED["nc"] = build_nc()
    return _CACHED["nc"]


def make_in_maps(logits, target):
    logits = np.ascontiguousarray(np.asarray(logits, dtype=np.float32))
    target = np.asarray(target)

    sel8 = np.zeros((128, G), dtype=ml_dtypes.bfloat16)
    for p in range(128):
        sel8[p, p // C] = 1.0
    ones128 = np.ones((128, 1), dtype=np.float32)
    ccol = (np.arange(128, dtype=np.float32) % C).reshape(128, 1)

    in_maps = []
    for n in range(N):
        t_flat = target[n].reshape(-1).astype(np.float32)
        # tpos layout: partition (8k + g) = t[g*FTOT + k*CHUNK : +CHUNK]
        tpos = np.transpose(
            t_flat.reshape(G, NCHUNK, CHUNK), (1, 0, 2)).reshape(128, CHUNK)
        in_maps.append({
            "x": logits[n].reshape(C, POS),
            "tpos": np.ascontiguousarray(tpos).astype(ml_dtypes.bfloat16),
            "sel8": sel8,
            "ones128": ones128,
            "ccol": ccol,
        })
    return in_maps


def combine(results):
    total = 0.0
    for r in results:
        total += np.asarray(r["out"], dtype=np.float64).sum()
    loss = -total / (float(N * POS) + SMOOTH)
    return np.float32(loss)


def kernel(logits, target, trace=False, **run_kwargs):
    nc = _get_nc()
    in_maps = make_in_maps(logits, target)
    res = run_bass_kernel_spmd(nc, in_maps, core_ids=list(range(8)),
                               trace=trace, **run_kwargs)
    out = combine(res.results)
    if trace:
        kernel.last_result = res
    return out


# revision 33
# speedup vs baseline: 1.4843x; 1.4843x over previous
"""AdaptiveFocalLoss on 8 TRN2 NeuronCores (Bass/Tile).

Data-parallel over batch N (8 images -> 8 cores). Per-core shard:
logits (16, 512*512) f32, target (512*512,) int.

Per-core device computation (positions P = 262144, C = 16):
  sweep layout: SBUF [128, F] with partition p = 16*g + c (g spatial group)
  expX  = exp(logits)                     (ACT, bf16 out)
  T_rep = target broadcast to channel partitions (PE matmul, PSUM)
  M     = (T_rep == c_partition) * expX   (DVE scalar_tensor_tensor)
  D     = sum_c expX    e_t = sum_c M     e'' = sum_c alpha_c * M
     -- all via PE "data-as-weights": lhsT = 128-col data block,
        rhs = Sel8 [128, 8] -> out[pos, group], full 128 partitions.
  lp = log e_t - log D (= log p_true);  a = exp(log e'' - log e_t) (= alpha_t)
  focal = a * (1 - p)^2 * (-lp);  loss = sum(focal)
Class counts: 16x 4x-mode is_equal masks + PE mask-reduce, AllReduce
across the 8 cores, alpha computed on-device -> weights of the e'' pass.
Host: sums per-core partial sums, divides by (numel + eps).
"""

import sys

sys.path.insert(0, "/opt/trn_rl_repo")

import numpy as np
import ml_dtypes

import concourse.bass as bass
import concourse.bacc as bacc
import concourse.tile as tile
from concourse import mybir
from concourse.bass_utils import run_bass_kernel_spmd

# ---- problem constants (hardcoded; kernel.py must be self-contained) ----
N, C, H, W = 8, 16, 512, 512
POS = H * W          # positions per core = 262144
G = 8                # spatial groups -> partition = 16*g + c
FTOT = POS // G      # free columns in (g,c) layout = 32768
CHUNK = 2048         # sweep chunk columns
NCHUNK = FTOT // CHUNK          # 16
SUB = 512            # PSUM bank free width (fp32)
SC_COLS = 8192       # columns per super-chunk (-> [128, 512] position tiles)
NSC = FTOT // SC_COLS           # 4
CHUNKS_PER_SC = SC_COLS // CHUNK  # 4
TW = 128             # tpos free width per chunk-row  (POS/128 = 2048)

GAMMA = 2.0
SMOOTH = 1e-8
ALPHA_SMOOTH = 0.1

FP32 = mybir.dt.float32
BF16 = mybir.dt.bfloat16
AX = mybir.AxisListType
OP = mybir.AluOpType
AF = mybir.ActivationFunctionType


def build_nc(compile_graph=True, use_collective=True, use_late=True):
    nc = bacc.Bacc("TRN2", target_bir_lowering=False, debug=False,
                   num_devices=8)

    x_ext = nc.declare_dram_parameter("x", [C, POS], FP32, isOutput=False)
    tpos_ext = nc.declare_dram_parameter("tpos", [128, POS // 128], BF16,
                                         isOutput=False)
    sel8_ext = nc.declare_dram_parameter("sel8", [128, G], BF16, isOutput=False)
    b8_ext = nc.declare_dram_parameter("b8", [G, 128], BF16, isOutput=False)
    ones_ext = nc.declare_dram_parameter("ones128", [128, 1], FP32,
                                         isOutput=False)
    onesb_ext = nc.declare_dram_parameter("onesb", [128, 1], BF16,
                                          isOutput=False)
    ccol_ext = nc.declare_dram_parameter("ccol", [128, 1], FP32, isOutput=False)
    out_ext = nc.declare_dram_parameter("out", [128, NSC], FP32, isOutput=True)
    dbg_ext = nc.declare_dram_parameter("dbg", [4, C], FP32, isOutput=True)

    # (g,c)-layout view of logits: partition p=(g,c) -> dram c*POS + g*FTOT + f
    x_gc = bass.AP(
        tensor=x_ext,
        offset=0,
        ap=[[FTOT, G], [POS, C], [1, FTOT]],
    )

    with tile.TileContext(nc) as tc:
        with (
            tc.tile_pool(name="singles", bufs=1) as singles,
            tc.tile_pool(name="xp", bufs=4) as xp,
            tc.tile_pool(name="exp", bufs=3) as exp_pool,
            tc.tile_pool(name="pos", bufs=2) as pos_pool,
            tc.tile_pool(name="late", bufs=NSC) as late_pool,
            tc.tile_pool(name="tiny", bufs=2) as tiny,
            tc.tile_pool(name="psA", bufs=2, space="PSUM") as psA,
            tc.tile_pool(name="psB", bufs=2, space="PSUM") as psB,
            tc.tile_pool(name="psT", bufs=1, space="PSUM") as psT,
            tc.tile_pool(name="dram", bufs=1, space="DRAM") as dram,
        ):
            # ---------------- constants / small inputs ----------------
            # DVE re-copies: hot-loop STT/LDW dependencies all ride the
            # single DVE semaphore (1 sync-wait slot per instruction).
            sel8_in = singles.tile([128, G], BF16)
            nc.sync.dma_start(out=sel8_in, in_=sel8_ext[:, :])
            sel8 = singles.tile([128, G], BF16)
            nc.vector.tensor_copy(out=sel8, in_=sel8_in)
            b8_in = singles.tile([G, 128], BF16)
            nc.sync.dma_start(out=b8_in, in_=b8_ext[:, :])
            b8 = singles.tile([G, 128], BF16)
            nc.vector.tensor_copy(out=b8, in_=b8_in)
            ones_in = singles.tile([128, 1], FP32)
            nc.sync.dma_start(out=ones_in, in_=ones_ext[:, :])
            ones128 = singles.tile([128, 1], FP32)
            nc.vector.tensor_copy(out=ones128, in_=ones_in)
            onesb_in = singles.tile([128, 1], BF16)
            nc.sync.dma_start(out=onesb_in, in_=onesb_ext[:, :])
            onesb = singles.tile([128, 1], BF16)
            nc.vector.tensor_copy(out=onesb, in_=onesb_in)
            ccol_in = singles.tile([128, 1], FP32)
            nc.sync.dma_start(out=ccol_in, in_=ccol_ext[:, :])
            ccol = singles.tile([128, 1], FP32)
            nc.vector.tensor_copy(out=ccol, in_=ccol_in)
            tpos = singles.tile([128, POS // 128], BF16)
            nc.sync.dma_start(out=tpos, in_=tpos_ext[:, :])

            # ------------- histogram: 4x-mode masks + PE reduce -------------
            # cnt_colps[:, c] += sum over partitions of (tpos == c) blocks
            cnt_colps = psB.tile([128, C], FP32, tag="EPP")
            for c in range(C):
                scr = tiny.tile([128, POS // 128], BF16, tag="hscr")
                nc.vector.tensor_scalar(
                    out=scr, in0=tpos, scalar1=float(c), scalar2=None,
                    op0=OP.is_equal,
                )
                nblk = (POS // 128) // 128
                for b in range(nblk):
                    nc.tensor.matmul(
                        cnt_colps[:, c:c + 1],
                        lhsT=scr[:, 128 * b:128 * (b + 1)], rhs=onesb,
                        start=(b == 0), stop=(b == nblk - 1),
                    )
            cnt_col = singles.tile([128, C], FP32)
            nc.vector.tensor_copy(out=cnt_col, in_=cnt_colps)
            cnt_ps = psA.tile([1, C], FP32, tag="D")
            nc.tensor.matmul(cnt_ps, lhsT=ones128, rhs=cnt_col, start=True,
                             stop=True)
            cnt_sb = singles.tile([1, C], FP32)
            nc.vector.tensor_copy(out=cnt_sb, in_=cnt_ps)

            cnt_g = singles.tile([1, C], FP32)
            if use_collective:
                cc_in = dram.tile([1, C], FP32)
                cc_out = dram.tile([1, C], FP32)
                nc.gpsimd.dma_start(out=cc_in[:], in_=cnt_sb)
                nc.gpsimd.collective_compute(
                    "AllReduce", OP.add,
                    replica_groups=[list(range(8))],
                    ins=[cc_in.opt()], outs=[cc_out.opt()],
                )
                nc.gpsimd.dma_start(out=cnt_g, in_=cc_out[:])
            else:
                nc.vector.tensor_scalar_mul(cnt_g, cnt_sb, 8.0)

            # ---------------- alpha from global counts ----------------
            wv = singles.tile([1, C], FP32)
            nc.vector.tensor_scalar(
                out=wv, in0=cnt_g, scalar1=1.0 / float(N * POS),
                scalar2=ALPHA_SMOOTH, op0=OP.mult, op1=OP.add,
            )
            nc.vector.reciprocal(out=wv, in_=wv)
            pres = singles.tile([1, C], FP32)
            nc.vector.tensor_scalar(
                out=pres, in0=cnt_g, scalar1=0.0, scalar2=None, op0=OP.is_gt,
            )
            wp = singles.tile([1, C], FP32)
            nc.vector.tensor_mul(wp, wv, pres)
            wsum = singles.tile([1, 1], FP32)
            nc.vector.tensor_reduce(out=wsum, in_=wp, axis=AX.X, op=OP.add)
            nc.vector.reciprocal(out=wsum, in_=wsum)
            alpha = singles.tile([1, C], FP32)
            nc.vector.tensor_scalar(
                out=alpha, in0=wp, scalar1=wsum, scalar2=None, op0=OP.mult,
            )
            omp = singles.tile([1, C], FP32)
            nc.vector.tensor_scalar(
                out=omp, in0=pres, scalar1=-1.0, scalar2=1.0,
                op0=OP.mult, op1=OP.add,
            )
            nc.vector.tensor_add(alpha, alpha, omp)

            nc.gpsimd.dma_start(out=dbg_ext[0:1, :], in_=cnt_g)
            nc.gpsimd.dma_start(out=dbg_ext[1:2, :], in_=alpha)

            # alpha -> [128,1] column (alpha_col[p] = alpha[p % 16])
            al_dram = dram.tile([1, C], FP32)
            nc.gpsimd.dma_start(out=al_dram[:], in_=alpha)
            alpha_in = singles.tile([128, 1], FP32)
            al_bcast = bass.AP(
                tensor=al_dram.tensor,
                offset=al_dram.offset,
                ap=[[0, G], [1, C], [1, 1]],
            )
            nc.gpsimd.dma_start(out=alpha_in, in_=al_bcast)
            alpha_col = singles.tile([128, 1], FP32)
            nc.vector.tensor_copy(out=alpha_col, in_=alpha_in)
            sel8a = singles.tile([128, G], BF16)
            nc.vector.tensor_scalar(
                out=sel8a, in0=sel8, scalar1=alpha_col, scalar2=None,
                op0=OP.mult,
            )

            # ---------------- main sweep ----------------
            m_all = singles.tile([128, FTOT], BF16)
            loss_col = singles.tile([128, NSC], FP32)
            d_tiles = {}
            e_tiles = {}
            le_tiles = {}
            f1_tiles = {}

            for k in range(NCHUNK):
                col0 = k * CHUNK
                cols = slice(col0, col0 + CHUNK)

                x_t = xp.tile([128, CHUNK], FP32, tag="x")
                # alternate DMA queue groups: HWDGE (0-7) / SWDGE (8-15)
                dma_eng = nc.sync if k % 2 == 0 else nc.gpsimd
                dma_eng.dma_start(out=x_t, in_=x_gc[:, :, cols])

                ex = exp_pool.tile([128, CHUNK], BF16, tag="ex")
                nc.scalar.activation(out=ex, in_=x_t, func=AF.Exp)

                s, j0 = divmod(k, CHUNKS_PER_SC)
                if j0 == 0:
                    d_tile = psA.tile([128, SUB], FP32, tag="D")
                    e_tile = psB.tile([128, SUB], FP32, tag="E")
                    d_tiles[s] = d_tile
                    e_tiles[s] = e_tile

                # T_rep via PE broadcast: trep_ps[16g+c, f] = tpos[8k+g, f].
                # matmul rhs must sit at partition 0 -> stage the 8 rows.
                tstage = tiny.tile([G, CHUNK], BF16, tag="tstage")
                nc.sync.dma_start(out=tstage, in_=tpos[G * k:G * k + G, :])
                for h in range(2):
                    hw = CHUNK // 2
                    hc = slice(col0 + h * hw, col0 + (h + 1) * hw)
                    hl = slice(h * hw, (h + 1) * hw)
                    trep_ps = psT.tile([128, hw], FP32, tag="trep")
                    for q in range(hw // SUB):
                        nc.tensor.matmul(
                            trep_ps[:, q * SUB:(q + 1) * SUB],
                            lhsT=b8,
                            rhs=tstage[:, h * hw + q * SUB:
                                       h * hw + (q + 1) * SUB],
                            start=True, stop=True,
                        )
                    nc.vector.scalar_tensor_tensor(
                        out=m_all[:, hc], in0=trep_ps, scalar=ccol,
                        in1=ex[:, hl], op0=OP.is_equal, op1=OP.mult,
                    )

                # D / e_t via data-as-weights matmuls
                for j in range(CHUNK // 128):
                    bb = j0 * (CHUNK // 128) + j  # block within super-chunk
                    blk = slice(col0 + j * 128, col0 + (j + 1) * 128)
                    nc.tensor.matmul(
                        d_tiles[s][:, 8 * bb:8 * bb + 8],
                        lhsT=ex[:, j * 128:(j + 1) * 128], rhs=sel8,
                        start=True, stop=True,
                    )
                    nc.tensor.matmul(
                        e_tiles[s][:, 8 * bb:8 * bb + 8],
                        lhsT=m_all[:, blk], rhs=sel8,
                        start=True, stop=True,
                    )

                if j0 == CHUNKS_PER_SC - 1:
                    # early epilogue for super-chunk s  (ACT: Exp/Ln/Square
                    # + affine Copy only -> single table set)
                    lD = pos_pool.tile([128, SUB], FP32, tag="lD")
                    nc.scalar.activation(out=lD, in_=d_tiles[s], func=AF.Ln)
                    le = late_pool.tile([128, SUB], FP32, tag="le")
                    nc.scalar.activation(out=le, in_=e_tiles[s], func=AF.Ln)
                    le_tiles[s] = le
                    lp = pos_pool.tile([128, SUB], FP32, tag="lp")
                    nc.vector.tensor_sub(lp, le, lD)
                    p = pos_pool.tile([128, SUB], FP32, tag="p")
                    nc.scalar.activation(out=p, in_=lp, func=AF.Exp)
                    u_t = pos_pool.tile([128, SUB], FP32, tag="u")
                    nc.scalar.activation(out=u_t, in_=p, func=AF.Copy,
                                         bias=1.0, scale=-1.0)
                    w_t = pos_pool.tile([128, SUB], FP32, tag="w")
                    nc.scalar.activation(out=w_t, in_=u_t, func=AF.Square)
                    f1 = late_pool.tile([128, SUB], FP32, tag="f1")
                    nc.vector.tensor_mul(f1, w_t, lp)
                    f1_tiles[s] = f1

            # ---------------- alpha-weighted pass + late epilogue ----------
            for s in range(NSC if use_late else 0):
                epp = psB.tile([128, SUB], FP32, tag="EPP")
                for bb in range(SC_COLS // 128):
                    col0 = s * SC_COLS + bb * 128
                    nc.tensor.matmul(
                        epp[:, 8 * bb:8 * bb + 8],
                        lhsT=m_all[:, col0:col0 + 128], rhs=sel8a,
                        start=True, stop=True,
                    )
                lepp = pos_pool.tile([128, SUB], FP32, tag="lepp")
                nc.scalar.activation(out=lepp, in_=epp, func=AF.Ln)
                la = pos_pool.tile([128, SUB], FP32, tag="la")
                nc.vector.tensor_sub(la, lepp, le_tiles[s])
                a_t = pos_pool.tile([128, SUB], FP32, tag="a")
                nc.scalar.activation(out=a_t, in_=la, func=AF.Exp)
                f2 = pos_pool.tile([128, SUB], FP32, tag="f2")
                nc.vector.tensor_mul(f2, f1_tiles[s], a_t)
                nc.vector.tensor_reduce(
                    out=loss_col[:, s:s + 1], in_=f2, axis=AX.X, op=OP.add)

            if not use_late:
                for s in range(NSC):
                    nc.vector.tensor_reduce(
                        out=loss_col[:, s:s + 1], in_=f1_tiles[s],
                        axis=AX.X, op=OP.add)

            nc.sync.dma_start(out=out_ext[:, :], in_=loss_col)

    if compile_graph:
        nc.compile()
    return nc


_CACHED = {}


def _get_nc():
    if "nc" not in _CACHED:
        _CACHED["nc"] = build_nc()
    return _CACHED["nc"]


def make_in_maps(logits, target):
    logits = np.ascontiguousarray(np.asarray(logits, dtype=np.float32))
    target = np.asarray(target)

    sel8 = np.zeros((128, G), dtype=ml_dtypes.bfloat16)
    for p in range(128):
        sel8[p, p // C] = 1.0
    b8 = np.zeros((G, 128), dtype=ml_dtypes.bfloat16)
    for m in range(128):
        b8[m // C, m] = 1.0
    ones128 = np.ones((128, 1), dtype=np.float32)
    onesb = np.ones((128, 1), dtype=ml_dtypes.bfloat16)
    ccol = (np.arange(128, dtype=np.float32) % C).reshape(128, 1)

    in_maps = []
    for n in range(N):
        t_flat = target[n].reshape(-1).astype(np.float32)
        # tpos layout: partition (8k + g) = t[g*FTOT + k*CHUNK : +CHUNK]
        tpos = np.transpose(
            t_flat.reshape(G, NCHUNK, CHUNK), (1, 0, 2)).reshape(128, CHUNK)
        in_maps.append({
            "x": logits[n].reshape(C, POS),
            "tpos": np.ascontiguousarray(tpos).astype(ml_dtypes.bfloat16),
            "sel8": sel8,
            "b8": b8,
            "ones128": ones128,
            "onesb": onesb,
            "ccol": ccol,
        })
    return in_maps


def combine(results):
    total = 0.0
    for r in results:
        total += np.asarray(r["out"], dtype=np.float64).sum()
    loss = -total / (float(N * POS) + SMOOTH)
    return np.float32(loss)


def kernel(logits, target, trace=False, **run_kwargs):
    nc = _get_nc()
    in_maps = make_in_maps(logits, target)
    res = run_bass_kernel_spmd(nc, in_maps, core_ids=list(range(8)),
                               trace=trace, **run_kwargs)
    out = combine(res.results)
    if trace:
        kernel.last_result = res
    return out


# revision 37
# speedup vs baseline: 1.7039x; 1.1480x over previous
"""AdaptiveFocalLoss on 8 TRN2 NeuronCores (Bass/Tile).

Data-parallel over batch N (8 images -> 8 cores). Per-core shard:
logits (16, 512*512) f32, target (512*512,) int.

Per-core device computation (positions P = 262144, C = 16):
  sweep layout: SBUF [128, F] with partition p = 16*g + c (g spatial group)
  expX  = exp(logits)                     (ACT, bf16 out)
  T_rep = target broadcast to channel partitions (PE matmul, PSUM)
  M     = (T_rep == c_partition) * expX   (DVE scalar_tensor_tensor)
  D     = sum_c expX    e_t = sum_c M     e'' = sum_c alpha_c * M
     -- all via PE "data-as-weights": lhsT = 128-col data block,
        rhs = Sel8 [128, 8] -> out[pos, group], full 128 partitions.
  lp = log e_t - log D (= log p_true);  a = exp(log e'' - log e_t) (= alpha_t)
  focal = a * (1 - p)^2 * (-lp);  loss = sum(focal)
Class counts: 16x 4x-mode is_equal masks + PE mask-reduce, AllReduce
across the 8 cores, alpha computed on-device -> weights of the e'' pass.
Host: sums per-core partial sums, divides by (numel + eps).
"""

import sys

sys.path.insert(0, "/opt/trn_rl_repo")

import numpy as np
import ml_dtypes

import concourse.bass as bass
import concourse.bacc as bacc
import concourse.tile as tile
from concourse import mybir
from concourse.bass_utils import run_bass_kernel_spmd

# ---- problem constants (hardcoded; kernel.py must be self-contained) ----
N, C, H, W = 8, 16, 512, 512
POS = H * W          # positions per core = 262144
G = 8                # spatial groups -> partition = 16*g + c
FTOT = POS // G      # free columns in (g,c) layout = 32768
CHUNK = 2048         # sweep chunk columns
NCHUNK = FTOT // CHUNK          # 16
SUB = 512            # PSUM bank free width (fp32)
SC_COLS = 8192       # columns per super-chunk (-> [128, 512] position tiles)
NSC = FTOT // SC_COLS           # 4
CHUNKS_PER_SC = SC_COLS // CHUNK  # 4
TW = 128             # tpos free width per chunk-row  (POS/128 = 2048)

GAMMA = 2.0
SMOOTH = 1e-8
ALPHA_SMOOTH = 0.1

FP32 = mybir.dt.float32
BF16 = mybir.dt.bfloat16
AX = mybir.AxisListType
OP = mybir.AluOpType
AF = mybir.ActivationFunctionType


def build_nc(compile_graph=True, use_collective=True, use_late=True):
    nc = bacc.Bacc("TRN2", target_bir_lowering=False, debug=False,
                   num_devices=8)

    x_ext = nc.declare_dram_parameter("x", [128, FTOT], FP32, isOutput=False)
    tpos_ext = nc.declare_dram_parameter("tpos", [128, POS // 128], BF16,
                                         isOutput=False)
    sel8_ext = nc.declare_dram_parameter("sel8", [128, G], BF16, isOutput=False)
    b8_ext = nc.declare_dram_parameter("b8", [G, 128], BF16, isOutput=False)
    ones_ext = nc.declare_dram_parameter("ones128", [128, 1], FP32,
                                         isOutput=False)
    onesb_ext = nc.declare_dram_parameter("onesb", [128, 1], BF16,
                                          isOutput=False)
    ccol_ext = nc.declare_dram_parameter("ccol", [128, 1], FP32, isOutput=False)
    out_ext = nc.declare_dram_parameter("out", [128, NSC], FP32, isOutput=True)
    dbg_ext = nc.declare_dram_parameter("dbg", [4, C], FP32, isOutput=True)


    with tile.TileContext(nc) as tc:
        with (
            tc.tile_pool(name="singles", bufs=1) as singles,
            tc.tile_pool(name="xp", bufs=3) as xp,
            tc.tile_pool(name="exp", bufs=3) as exp_pool,
            tc.tile_pool(name="pos", bufs=2) as pos_pool,
            tc.tile_pool(name="late", bufs=NSC) as late_pool,
            tc.tile_pool(name="tiny", bufs=2) as tiny,
            tc.tile_pool(name="psA", bufs=2, space="PSUM") as psA,
            tc.tile_pool(name="psB", bufs=2, space="PSUM") as psB,
            tc.tile_pool(name="psT", bufs=1, space="PSUM") as psT,
            tc.tile_pool(name="dram", bufs=1, space="DRAM") as dram,
        ):
            # ---------------- constants / small inputs ----------------
            # DVE re-copies: hot-loop STT/LDW dependencies all ride the
            # single DVE semaphore (1 sync-wait slot per instruction).
            sel8_in = singles.tile([128, G], BF16)
            nc.sync.dma_start(out=sel8_in, in_=sel8_ext[:, :])
            sel8 = singles.tile([128, G], BF16)
            nc.vector.tensor_copy(out=sel8, in_=sel8_in)
            b8_in = singles.tile([G, 128], BF16)
            nc.sync.dma_start(out=b8_in, in_=b8_ext[:, :])
            b8 = singles.tile([G, 128], BF16)
            nc.vector.tensor_copy(out=b8, in_=b8_in)
            ones_in = singles.tile([128, 1], FP32)
            nc.sync.dma_start(out=ones_in, in_=ones_ext[:, :])
            ones128 = singles.tile([128, 1], FP32)
            nc.vector.tensor_copy(out=ones128, in_=ones_in)
            onesb_in = singles.tile([128, 1], BF16)
            nc.sync.dma_start(out=onesb_in, in_=onesb_ext[:, :])
            onesb = singles.tile([128, 1], BF16)
            nc.vector.tensor_copy(out=onesb, in_=onesb_in)
            ccol_in = singles.tile([128, 1], FP32)
            nc.sync.dma_start(out=ccol_in, in_=ccol_ext[:, :])
            ccol = singles.tile([128, 1], FP32)
            nc.vector.tensor_copy(out=ccol, in_=ccol_in)
            tpos = singles.tile([128, POS // 128], BF16)
            nc.sync.dma_start(out=tpos, in_=tpos_ext[:, :])

            # ------------- histogram: 4x-mode masks + PE reduce -------------
            # cnt_colps[:, c] += sum over partitions of (tpos == c) blocks
            cnt_colps = psB.tile([128, C], FP32, tag="EPP")
            for c in range(C):
                scr = tiny.tile([128, POS // 128], BF16, tag="hscr")
                nc.vector.tensor_scalar(
                    out=scr, in0=tpos, scalar1=float(c), scalar2=None,
                    op0=OP.is_equal,
                )
                nblk = (POS // 128) // 128
                for b in range(nblk):
                    nc.tensor.matmul(
                        cnt_colps[:, c:c + 1],
                        lhsT=scr[:, 128 * b:128 * (b + 1)], rhs=onesb,
                        start=(b == 0), stop=(b == nblk - 1),
                    )
            cnt_col = singles.tile([128, C], FP32)
            nc.vector.tensor_copy(out=cnt_col, in_=cnt_colps)
            cnt_ps = psA.tile([1, C], FP32, tag="D")
            nc.tensor.matmul(cnt_ps, lhsT=ones128, rhs=cnt_col, start=True,
                             stop=True)
            cnt_sb = singles.tile([1, C], FP32)
            nc.vector.tensor_copy(out=cnt_sb, in_=cnt_ps)

            cnt_g = singles.tile([1, C], FP32)
            if use_collective:
                cc_in = dram.tile([1, C], FP32)
                cc_out = dram.tile([1, C], FP32)
                nc.gpsimd.dma_start(out=cc_in[:], in_=cnt_sb)
                nc.gpsimd.collective_compute(
                    "AllReduce", OP.add,
                    replica_groups=[list(range(8))],
                    ins=[cc_in.opt()], outs=[cc_out.opt()],
                )
                nc.gpsimd.dma_start(out=cnt_g, in_=cc_out[:])
            else:
                nc.vector.tensor_scalar_mul(cnt_g, cnt_sb, 8.0)

            # ---------------- alpha from global counts ----------------
            wv = singles.tile([1, C], FP32)
            nc.vector.tensor_scalar(
                out=wv, in0=cnt_g, scalar1=1.0 / float(N * POS),
                scalar2=ALPHA_SMOOTH, op0=OP.mult, op1=OP.add,
            )
            nc.vector.reciprocal(out=wv, in_=wv)
            pres = singles.tile([1, C], FP32)
            nc.vector.tensor_scalar(
                out=pres, in0=cnt_g, scalar1=0.0, scalar2=None, op0=OP.is_gt,
            )
            wp = singles.tile([1, C], FP32)
            nc.vector.tensor_mul(wp, wv, pres)
            wsum = singles.tile([1, 1], FP32)
            nc.vector.tensor_reduce(out=wsum, in_=wp, axis=AX.X, op=OP.add)
            nc.vector.reciprocal(out=wsum, in_=wsum)
            alpha = singles.tile([1, C], FP32)
            nc.vector.tensor_scalar(
                out=alpha, in0=wp, scalar1=wsum, scalar2=None, op0=OP.mult,
            )
            omp = singles.tile([1, C], FP32)
            nc.vector.tensor_scalar(
                out=omp, in0=pres, scalar1=-1.0, scalar2=1.0,
                op0=OP.mult, op1=OP.add,
            )
            nc.vector.tensor_add(alpha, alpha, omp)

            nc.gpsimd.dma_start(out=dbg_ext[0:1, :], in_=cnt_g)
            nc.gpsimd.dma_start(out=dbg_ext[1:2, :], in_=alpha)

            # alpha -> [128,1] column (alpha_col[p] = alpha[p % 16])
            al_dram = dram.tile([1, C], FP32)
            nc.gpsimd.dma_start(out=al_dram[:], in_=alpha)
            alpha_in = singles.tile([128, 1], FP32)
            al_bcast = bass.AP(
                tensor=al_dram.tensor,
                offset=al_dram.offset,
                ap=[[0, G], [1, C], [1, 1]],
            )
            nc.gpsimd.dma_start(out=alpha_in, in_=al_bcast)
            alpha_col = singles.tile([128, 1], FP32)
            nc.vector.tensor_copy(out=alpha_col, in_=alpha_in)
            sel8a = singles.tile([128, G], BF16)
            nc.vector.tensor_scalar(
                out=sel8a, in0=sel8, scalar1=alpha_col, scalar2=None,
                op0=OP.mult,
            )

            # ---------------- main sweep ----------------
            m_all = singles.tile([128, FTOT], BF16)
            loss_col = singles.tile([128, NSC], FP32)
            d_tiles = {}
            e_tiles = {}
            le_tiles = {}
            f1_tiles = {}

            for k in range(NCHUNK):
                col0 = k * CHUNK
                cols = slice(col0, col0 + CHUNK)

                x_t = xp.tile([128, CHUNK], FP32, tag="x")
                # alternate DMA queue groups: HWDGE (0-7) / SWDGE (8-15)
                dma_eng = nc.sync if k % 2 == 0 else nc.gpsimd
                dma_eng.dma_start(out=x_t, in_=x_ext[:, cols])

                ex = exp_pool.tile([128, CHUNK], BF16, tag="ex")
                nc.scalar.activation(out=ex, in_=x_t, func=AF.Exp)

                s, j0 = divmod(k, CHUNKS_PER_SC)
                if j0 == 0:
                    d_tile = psA.tile([128, SUB], FP32, tag="D")
                    e_tile = psB.tile([128, SUB], FP32, tag="E")
                    d_tiles[s] = d_tile
                    e_tiles[s] = e_tile

                # T_rep via PE broadcast: trep_ps[16g+c, f] = tpos[8k+g, f].
                # matmul rhs must sit at partition 0 -> stage the 8 rows.
                tstage = tiny.tile([G, CHUNK], BF16, tag="tstage")
                nc.sync.dma_start(out=tstage, in_=tpos[G * k:G * k + G, :])
                for h in range(2):
                    hw = CHUNK // 2
                    hc = slice(col0 + h * hw, col0 + (h + 1) * hw)
                    hl = slice(h * hw, (h + 1) * hw)
                    trep_ps = psT.tile([128, hw], FP32, tag="trep")
                    for q in range(hw // SUB):
                        nc.tensor.matmul(
                            trep_ps[:, q * SUB:(q + 1) * SUB],
                            lhsT=b8,
                            rhs=tstage[:, h * hw + q * SUB:
                                       h * hw + (q + 1) * SUB],
                            start=True, stop=True,
                        )
                    nc.vector.scalar_tensor_tensor(
                        out=m_all[:, hc], in0=trep_ps, scalar=ccol,
                        in1=ex[:, hl], op0=OP.is_equal, op1=OP.mult,
                    )

                # D / e_t via data-as-weights matmuls
                for j in range(CHUNK // 128):
                    bb = j0 * (CHUNK // 128) + j  # block within super-chunk
                    blk = slice(col0 + j * 128, col0 + (j + 1) * 128)
                    nc.tensor.matmul(
                        d_tiles[s][:, 8 * bb:8 * bb + 8],
                        lhsT=ex[:, j * 128:(j + 1) * 128], rhs=sel8,
                        start=True, stop=True,
                    )
                    nc.tensor.matmul(
                        e_tiles[s][:, 8 * bb:8 * bb + 8],
                        lhsT=m_all[:, blk], rhs=sel8,
                        start=True, stop=True,
                    )

                if j0 == CHUNKS_PER_SC - 1:
                    # early epilogue for super-chunk s  (ACT: Exp/Ln/Square
                    # + affine Copy only -> single table set)
                    lD = pos_pool.tile([128, SUB], FP32, tag="lD")
                    nc.scalar.activation(out=lD, in_=d_tiles[s], func=AF.Ln)
                    le = late_pool.tile([128, SUB], FP32, tag="le")
                    nc.scalar.activation(out=le, in_=e_tiles[s], func=AF.Ln)
                    le_tiles[s] = le
                    lp = pos_pool.tile([128, SUB], FP32, tag="lp")
                    nc.vector.tensor_sub(lp, le, lD)
                    p = pos_pool.tile([128, SUB], FP32, tag="p")
                    nc.scalar.activation(out=p, in_=lp, func=AF.Exp)
                    u_t = pos_pool.tile([128, SUB], FP32, tag="u")
                    nc.scalar.activation(out=u_t, in_=p, func=AF.Copy,
                                         bias=1.0, scale=-1.0)
                    w_t = pos_pool.tile([128, SUB], FP32, tag="w")
                    nc.scalar.activation(out=w_t, in_=u_t, func=AF.Square)
                    f1 = late_pool.tile([128, SUB], FP32, tag="f1")
                    nc.vector.tensor_mul(f1, w_t, lp)
                    f1_tiles[s] = f1

            # ---------------- alpha-weighted pass + late epilogue ----------
            for s in range(NSC if use_late else 0):
                epp = psB.tile([128, SUB], FP32, tag="EPP")
                for bb in range(SC_COLS // 128):
                    col0 = s * SC_COLS + bb * 128
                    nc.tensor.matmul(
                        epp[:, 8 * bb:8 * bb + 8],
                        lhsT=m_all[:, col0:col0 + 128], rhs=sel8a,
                        start=True, stop=True,
                    )
                lepp = pos_pool.tile([128, SUB], FP32, tag="lepp")
                nc.scalar.activation(out=lepp, in_=epp, func=AF.Ln)
                la = pos_pool.tile([128, SUB], FP32, tag="la")
                nc.vector.tensor_sub(la, lepp, le_tiles[s])
                a_t = pos_pool.tile([128, SUB], FP32, tag="a")
                nc.scalar.activation(out=a_t, in_=la, func=AF.Exp)
                f2 = pos_pool.tile([128, SUB], FP32, tag="f2")
                nc.vector.tensor_mul(f2, f1_tiles[s], a_t)
                nc.vector.tensor_reduce(
                    out=loss_col[:, s:s + 1], in_=f2, axis=AX.X, op=OP.add)

            if not use_late:
                for s in range(NSC):
                    nc.vector.tensor_reduce(
                        out=loss_col[:, s:s + 1], in_=f1_tiles[s],
                        axis=AX.X, op=OP.add)

            nc.sync.dma_start(out=out_ext[:, :], in_=loss_col)

    if compile_graph:
        nc.compile()
    return nc


_CACHED = {}


def _get_nc():
    if "nc" not in _CACHED:
        _CACHED["nc"] = build_nc()
    return _CACHED["nc"]


def make_in_maps(logits, target):
    logits = np.ascontiguousarray(np.asarray(logits, dtype=np.float32))
    target = np.asarray(target)

    sel8 = np.zeros((128, G), dtype=ml_dtypes.bfloat16)
    for p in range(128):
        sel8[p, p // C] = 1.0
    b8 = np.zeros((G, 128), dtype=ml_dtypes.bfloat16)
    for m in range(128):
        b8[m // C, m] = 1.0
    ones128 = np.ones((128, 1), dtype=np.float32)
    onesb = np.ones((128, 1), dtype=ml_dtypes.bfloat16)
    ccol = (np.arange(128, dtype=np.float32) % C).reshape(128, 1)

    in_maps = []
    for n in range(N):
        t_flat = target[n].reshape(-1).astype(np.float32)
        # logits in (g,c)-layout: row 16g+c = logits[c, g*FTOT : (g+1)*FTOT]
        x128 = np.ascontiguousarray(np.transpose(
            logits[n].reshape(C, G, FTOT), (1, 0, 2)).reshape(128, FTOT))
        # tpos layout: partition (8k + g) = t[g*FTOT + k*2048 : +2048]
        tpos = np.transpose(
            t_flat.reshape(G, 16, 2048), (1, 0, 2)).reshape(128, 2048)
        in_maps.append({
            "x": x128,
            "tpos": np.ascontiguousarray(tpos).astype(ml_dtypes.bfloat16),
            "sel8": sel8,
            "b8": b8,
            "ones128": ones128,
            "onesb": onesb,
            "ccol": ccol,
        })
    return in_maps


def combine(results):
    total = 0.0
    for r in results:
        total += np.asarray(r["out"], dtype=np.float64).sum()
    loss = -total / (float(N * POS) + SMOOTH)
    return np.float32(loss)


def kernel(logits, target, trace=False, **run_kwargs):
    nc = _get_nc()
    in_maps = make_in_maps(logits, target)
    res = run_bass_kernel_spmd(nc, in_maps, core_ids=list(range(8)),
                               trace=trace, **run_kwargs)
    out = combine(res.results)
    if trace:
        kernel.last_result = res
    return out


# revision 38
# speedup vs baseline: 1.9906x; 1.1682x over previous
"""AdaptiveFocalLoss on 8 TRN2 NeuronCores (Bass/Tile).

Data-parallel over batch N (8 images -> 8 cores). Per-core shard:
logits (16, 512*512) f32, target (512*512,) int.

Per-core device computation (positions P = 262144, C = 16):
  sweep layout: SBUF [128, F] with partition p = 16*g + c (g spatial group)
  expX  = exp(logits)                     (ACT, bf16 out)
  T_rep = target broadcast to channel partitions (PE matmul, PSUM)
  M     = (T_rep == c_partition) * expX   (DVE scalar_tensor_tensor)
  D     = sum_c expX    e_t = sum_c M     e'' = sum_c alpha_c * M
     -- all via PE "data-as-weights": lhsT = 128-col data block,
        rhs = Sel8 [128, 8] -> out[pos, group], full 128 partitions.
  lp = log e_t - log D (= log p_true);  a = exp(log e'' - log e_t) (= alpha_t)
  focal = a * (1 - p)^2 * (-lp);  loss = sum(focal)
Class counts: 16x 4x-mode is_equal masks + PE mask-reduce, AllReduce
across the 8 cores, alpha computed on-device -> weights of the e'' pass.
Host: sums per-core partial sums, divides by (numel + eps).
"""

import sys

sys.path.insert(0, "/opt/trn_rl_repo")

import numpy as np
import ml_dtypes

import concourse.bass as bass
import concourse.bacc as bacc
import concourse.tile as tile
from concourse import mybir
from concourse.bass_utils import run_bass_kernel_spmd

# ---- problem constants (hardcoded; kernel.py must be self-contained) ----
N, C, H, W = 8, 16, 512, 512
POS = H * W          # positions per core = 262144
G = 8                # spatial groups -> partition = 16*g + c
FTOT = POS // G      # free columns in (g,c) layout = 32768
CHUNK = 2048         # sweep chunk columns
NCHUNK = FTOT // CHUNK          # 16
SUB = 512            # PSUM bank free width (fp32)
SC_COLS = 8192       # columns per super-chunk (-> [128, 512] position tiles)
NSC = FTOT // SC_COLS           # 4
CHUNKS_PER_SC = SC_COLS // CHUNK  # 4
TW = 128             # tpos free width per chunk-row  (POS/128 = 2048)

GAMMA = 2.0
SMOOTH = 1e-8
ALPHA_SMOOTH = 0.1

FP32 = mybir.dt.float32
BF16 = mybir.dt.bfloat16
AX = mybir.AxisListType
OP = mybir.AluOpType
AF = mybir.ActivationFunctionType


def build_nc(compile_graph=True, use_collective=True, use_late=True):
    nc = bacc.Bacc("TRN2", target_bir_lowering=False, debug=False,
                   num_devices=8)

    x_ext = nc.declare_dram_parameter("x", [128, FTOT], FP32, isOutput=False)
    tpos_ext = nc.declare_dram_parameter("tpos", [128, POS // 128], BF16,
                                         isOutput=False)
    sel8_ext = nc.declare_dram_parameter("sel8", [128, G], BF16, isOutput=False)
    b8_ext = nc.declare_dram_parameter("b8", [G, 128], BF16, isOutput=False)
    ones_ext = nc.declare_dram_parameter("ones128", [128, 1], FP32,
                                         isOutput=False)
    onesb_ext = nc.declare_dram_parameter("onesb", [128, 1], BF16,
                                          isOutput=False)
    ccol_ext = nc.declare_dram_parameter("ccol", [128, 1], FP32, isOutput=False)
    out_ext = nc.declare_dram_parameter("out", [128, NSC], FP32, isOutput=True)
    dbg_ext = nc.declare_dram_parameter("dbg", [4, C], FP32, isOutput=True)


    with tile.TileContext(nc) as tc:
        with (
            tc.tile_pool(name="singles", bufs=1) as singles,
            tc.tile_pool(name="xp", bufs=4) as xp,
            tc.tile_pool(name="exp", bufs=3) as exp_pool,
            tc.tile_pool(name="pos", bufs=2) as pos_pool,
            tc.tile_pool(name="late", bufs=NSC) as late_pool,
            tc.tile_pool(name="tiny", bufs=2) as tiny,
            tc.tile_pool(name="psA", bufs=1, space="PSUM") as psA,
            tc.tile_pool(name="psE", bufs=1, space="PSUM") as psE,
            tc.tile_pool(name="psB", bufs=2, space="PSUM") as psB,
            tc.tile_pool(name="psT", bufs=2, space="PSUM") as psT,
            tc.tile_pool(name="dram", bufs=1, space="DRAM") as dram,
        ):
            # ---------------- constants / small inputs ----------------
            # DVE re-copies: hot-loop STT/LDW dependencies all ride the
            # single DVE semaphore (1 sync-wait slot per instruction).
            sel8_in = singles.tile([128, G], BF16)
            nc.sync.dma_start(out=sel8_in, in_=sel8_ext[:, :])
            sel8 = singles.tile([128, G], BF16)
            nc.vector.tensor_copy(out=sel8, in_=sel8_in)
            b8_in = singles.tile([G, 128], BF16)
            nc.sync.dma_start(out=b8_in, in_=b8_ext[:, :])
            b8 = singles.tile([G, 128], BF16)
            nc.vector.tensor_copy(out=b8, in_=b8_in)
            ones_in = singles.tile([128, 1], FP32)
            nc.sync.dma_start(out=ones_in, in_=ones_ext[:, :])
            ones128 = singles.tile([128, 1], FP32)
            nc.vector.tensor_copy(out=ones128, in_=ones_in)
            onesb_in = singles.tile([128, 1], BF16)
            nc.sync.dma_start(out=onesb_in, in_=onesb_ext[:, :])
            onesb = singles.tile([128, 1], BF16)
            nc.vector.tensor_copy(out=onesb, in_=onesb_in)
            ccol_in = singles.tile([128, 1], FP32)
            nc.sync.dma_start(out=ccol_in, in_=ccol_ext[:, :])
            ccol = singles.tile([128, 1], FP32)
            nc.vector.tensor_copy(out=ccol, in_=ccol_in)
            tpos = singles.tile([128, POS // 128], BF16)
            nc.gpsimd.dma_start(out=tpos, in_=tpos_ext[:, :])

            # ------------- histogram: 4x-mode masks + PE reduce -------------
            # cnt_colps[:, c] += sum over partitions of (tpos == c) blocks
            cnt_colps = psB.tile([128, C], FP32, tag="EPP")
            for c in range(C):
                scr = tiny.tile([128, POS // 128], BF16, tag="hscr")
                nc.vector.tensor_scalar(
                    out=scr, in0=tpos, scalar1=float(c), scalar2=None,
                    op0=OP.is_equal,
                )
                nblk = (POS // 128) // 128
                for b in range(nblk):
                    nc.tensor.matmul(
                        cnt_colps[:, c:c + 1],
                        lhsT=scr[:, 128 * b:128 * (b + 1)], rhs=onesb,
                        start=(b == 0), stop=(b == nblk - 1),
                    )
            cnt_col = singles.tile([128, C], FP32)
            nc.vector.tensor_copy(out=cnt_col, in_=cnt_colps)
            cnt_ps = psA.tile([1, C], FP32, tag="D")
            nc.tensor.matmul(cnt_ps, lhsT=ones128, rhs=cnt_col, start=True,
                             stop=True)
            cnt_sb = singles.tile([1, C], FP32)
            nc.vector.tensor_copy(out=cnt_sb, in_=cnt_ps)

            cnt_g = singles.tile([1, C], FP32)
            if use_collective:
                cc_in = dram.tile([1, C], FP32)
                cc_out = dram.tile([1, C], FP32)
                nc.gpsimd.dma_start(out=cc_in[:], in_=cnt_sb)
                nc.gpsimd.collective_compute(
                    "AllReduce", OP.add,
                    replica_groups=[list(range(8))],
                    ins=[cc_in.opt()], outs=[cc_out.opt()],
                )
                nc.gpsimd.dma_start(out=cnt_g, in_=cc_out[:])
            else:
                nc.vector.tensor_scalar_mul(cnt_g, cnt_sb, 8.0)

            # ---------------- alpha from global counts ----------------
            wv = singles.tile([1, C], FP32)
            nc.vector.tensor_scalar(
                out=wv, in0=cnt_g, scalar1=1.0 / float(N * POS),
                scalar2=ALPHA_SMOOTH, op0=OP.mult, op1=OP.add,
            )
            nc.vector.reciprocal(out=wv, in_=wv)
            pres = singles.tile([1, C], FP32)
            nc.vector.tensor_scalar(
                out=pres, in0=cnt_g, scalar1=0.0, scalar2=None, op0=OP.is_gt,
            )
            wp = singles.tile([1, C], FP32)
            nc.vector.tensor_mul(wp, wv, pres)
            wsum = singles.tile([1, 1], FP32)
            nc.vector.tensor_reduce(out=wsum, in_=wp, axis=AX.X, op=OP.add)
            nc.vector.reciprocal(out=wsum, in_=wsum)
            alpha = singles.tile([1, C], FP32)
            nc.vector.tensor_scalar(
                out=alpha, in0=wp, scalar1=wsum, scalar2=None, op0=OP.mult,
            )
            omp = singles.tile([1, C], FP32)
            nc.vector.tensor_scalar(
                out=omp, in0=pres, scalar1=-1.0, scalar2=1.0,
                op0=OP.mult, op1=OP.add,
            )
            nc.vector.tensor_add(alpha, alpha, omp)

            nc.gpsimd.dma_start(out=dbg_ext[0:1, :], in_=cnt_g)
            nc.gpsimd.dma_start(out=dbg_ext[1:2, :], in_=alpha)

            # alpha -> [128,1] column (alpha_col[p] = alpha[p % 16])
            al_dram = dram.tile([1, C], FP32)
            nc.gpsimd.dma_start(out=al_dram[:], in_=alpha)
            alpha_in = singles.tile([128, 1], FP32)
            al_bcast = bass.AP(
                tensor=al_dram.tensor,
                offset=al_dram.offset,
                ap=[[0, G], [1, C], [1, 1]],
            )
            nc.gpsimd.dma_start(out=alpha_in, in_=al_bcast)
            alpha_col = singles.tile([128, 1], FP32)
            nc.vector.tensor_copy(out=alpha_col, in_=alpha_in)
            sel8a = singles.tile([128, G], BF16)
            nc.vector.tensor_scalar(
                out=sel8a, in0=sel8, scalar1=alpha_col, scalar2=None,
                op0=OP.mult,
            )

            # ---------------- main sweep ----------------
            m_all = singles.tile([128, FTOT], BF16)
            loss_col = singles.tile([128, NSC], FP32)
            d_tiles = {}
            e_tiles = {}
            le_tiles = {}
            f1_tiles = {}

            for k in range(NCHUNK):
                col0 = k * CHUNK
                cols = slice(col0, col0 + CHUNK)

                x_t = xp.tile([128, CHUNK], FP32, tag="x")
                # alternate DMA queue groups: HWDGE (0-7) / SWDGE (8-15)
                dma_eng = nc.sync if k % 2 == 0 else nc.gpsimd
                dma_eng.dma_start(out=x_t, in_=x_ext[:, cols])

                ex = exp_pool.tile([128, CHUNK], BF16, tag="ex")
                nc.scalar.activation(out=ex, in_=x_t, func=AF.Exp)

                s, j0 = divmod(k, CHUNKS_PER_SC)
                if j0 == 0:
                    d_tile = psA.tile([128, SUB], FP32, tag="D")
                    e_tile = psE.tile([128, SUB], FP32, tag="E")
                    d_tiles[s] = d_tile
                    e_tiles[s] = e_tile

                # T_rep via PE broadcast: trep_ps[16g+c, f] = tpos[8k+g, f].
                # matmul rhs must sit at partition 0 -> stage the 8 rows.
                tstage = tiny.tile([G, CHUNK], BF16, tag="tstage")
                nc.sync.dma_start(out=tstage, in_=tpos[G * k:G * k + G, :])
                for h in range(2):
                    hw = CHUNK // 2
                    hc = slice(col0 + h * hw, col0 + (h + 1) * hw)
                    hl = slice(h * hw, (h + 1) * hw)
                    trep_ps = psT.tile([128, hw], FP32, tag="trep")
                    for q in range(hw // SUB):
                        nc.tensor.matmul(
                            trep_ps[:, q * SUB:(q + 1) * SUB],
                            lhsT=b8,
                            rhs=tstage[:, h * hw + q * SUB:
                                       h * hw + (q + 1) * SUB],
                            start=True, stop=True,
                        )
                    nc.vector.scalar_tensor_tensor(
                        out=m_all[:, hc], in0=trep_ps, scalar=ccol,
                        in1=ex[:, hl], op0=OP.is_equal, op1=OP.mult,
                    )

                # D / e_t via data-as-weights matmuls
                for j in range(CHUNK // 128):
                    bb = j0 * (CHUNK // 128) + j  # block within super-chunk
                    blk = slice(col0 + j * 128, col0 + (j + 1) * 128)
                    nc.tensor.matmul(
                        d_tiles[s][:, 8 * bb:8 * bb + 8],
                        lhsT=ex[:, j * 128:(j + 1) * 128], rhs=sel8,
                        start=True, stop=True,
                    )
                    nc.tensor.matmul(
                        e_tiles[s][:, 8 * bb:8 * bb + 8],
                        lhsT=m_all[:, blk], rhs=sel8,
                        start=True, stop=True,
                    )

                if j0 == CHUNKS_PER_SC - 1:
                    # early epilogue for super-chunk s  (ACT: Exp/Ln/Square
                    # + affine Copy only -> single table set)
                    lD = pos_pool.tile([128, SUB], FP32, tag="lD")
                    nc.scalar.activation(out=lD, in_=d_tiles[s], func=AF.Ln)
                    le = late_pool.tile([128, SUB], FP32, tag="le")
                    nc.scalar.activation(out=le, in_=e_tiles[s], func=AF.Ln)
                    le_tiles[s] = le
                    lp = pos_pool.tile([128, SUB], FP32, tag="lp")
                    nc.vector.tensor_sub(lp, le, lD)
                    p = pos_pool.tile([128, SUB], FP32, tag="p")
                    nc.scalar.activation(out=p, in_=lp, func=AF.Exp)
                    u_t = pos_pool.tile([128, SUB], FP32, tag="u")
                    nc.vector.tensor_scalar(
                        out=u_t, in0=p, scalar1=-1.0, scalar2=1.0,
                        op0=OP.mult, op1=OP.add)
                    w_t = pos_pool.tile([128, SUB], FP32, tag="w")
                    nc.vector.tensor_mul(w_t, u_t, u_t)
                    f1 = late_pool.tile([128, SUB], FP32, tag="f1")
                    nc.vector.tensor_mul(f1, w_t, lp)
                    f1_tiles[s] = f1

            # ---------------- alpha-weighted pass + late epilogue ----------
            for s in range(NSC if use_late else 0):
                epp = psB.tile([128, SUB], FP32, tag="EPP")
                for bb in range(SC_COLS // 128):
                    col0 = s * SC_COLS + bb * 128
                    nc.tensor.matmul(
                        epp[:, 8 * bb:8 * bb + 8],
                        lhsT=m_all[:, col0:col0 + 128], rhs=sel8a,
                        start=True, stop=True,
                    )
                lepp = pos_pool.tile([128, SUB], FP32, tag="lepp")
                nc.scalar.activation(out=lepp, in_=epp, func=AF.Ln)
                la = pos_pool.tile([128, SUB], FP32, tag="la")
                nc.vector.tensor_sub(la, lepp, le_tiles[s])
                a_t = pos_pool.tile([128, SUB], FP32, tag="a")
                nc.scalar.activation(out=a_t, in_=la, func=AF.Exp)
                f2 = pos_pool.tile([128, SUB], FP32, tag="f2")
                nc.vector.tensor_mul(f2, f1_tiles[s], a_t)
                nc.vector.tensor_reduce(
                    out=loss_col[:, s:s + 1], in_=f2, axis=AX.X, op=OP.add)

            if not use_late:
                for s in range(NSC):
                    nc.vector.tensor_reduce(
                        out=loss_col[:, s:s + 1], in_=f1_tiles[s],
                        axis=AX.X, op=OP.add)

            nc.sync.dma_start(out=out_ext[:, :], in_=loss_col)

    if compile_graph:
        nc.compile()
    return nc


_CACHED = {}


def _get_nc():
    if "nc" not in _CACHED:
        _CACHED["nc"] = build_nc()
    return _CACHED["nc"]


def make_in_maps(logits, target):
    logits = np.ascontiguousarray(np.asarray(logits, dtype=np.float32))
    target = np.asarray(target)

    sel8 = np.zeros((128, G), dtype=ml_dtypes.bfloat16)
    for p in range(128):
        sel8[p, p // C] = 1.0
    b8 = np.zeros((G, 128), dtype=ml_dtypes.bfloat16)
    for m in range(128):
        b8[m // C, m] = 1.0
    ones128 = np.ones((128, 1), dtype=np.float32)
    onesb = np.ones((128, 1), dtype=ml_dtypes.bfloat16)
    ccol = (np.arange(128, dtype=np.float32) % C).reshape(128, 1)

    in_maps = []
    for n in range(N):
        t_flat = target[n].reshape(-1).astype(np.float32)
        # logits in (g,c)-layout: row 16g+c = logits[c, g*FTOT : (g+1)*FTOT]
        x128 = np.ascontiguousarray(np.transpose(
            logits[n].reshape(C, G, FTOT), (1, 0, 2)).reshape(128, FTOT))
        # tpos layout: partition (8k + g) = t[g*FTOT + k*2048 : +2048]
        tpos = np.transpose(
            t_flat.reshape(G, 16, 2048), (1, 0, 2)).reshape(128, 2048)
        in_maps.append({
            "x": x128,
            "tpos": np.ascontiguousarray(tpos).astype(ml_dtypes.bfloat16),
            "sel8": sel8,
            "b8": b8,
            "ones128": ones128,
            "onesb": onesb,
            "ccol": ccol,
        })
    return in_maps


def combine(results):
    total = 0.0
    for r in results:
        total += np.asarray(r["out"], dtype=np.float64).sum()
    loss = -total / (float(N * POS) + SMOOTH)
    return np.float32(loss)


def kernel(logits, target, trace=False, **run_kwargs):
    nc = _get_nc()
    in_maps = make_in_maps(logits, target)
    res = run_bass_kernel_spmd(nc, in_maps, core_ids=list(range(8)),
                               trace=trace, **run_kwargs)
    out = combine(res.results)
    if trace:
        kernel.last_result = res
    return out


# revision 39
# speedup vs baseline: 2.0139x; 1.0117x over previous
"""AdaptiveFocalLoss on 8 TRN2 NeuronCores (Bass/Tile).

Data-parallel over batch N (8 images -> 8 cores). Per-core shard:
logits (16, 512*512) f32, target (512*512,) int.

Per-core device computation (positions P = 262144, C = 16):
  sweep layout: SBUF [128, F] with partition p = 16*g + c (g spatial group)
  expX  = exp(logits)                     (ACT, bf16 out)
  T_rep = target broadcast to channel partitions (PE matmul, PSUM)
  M     = (T_rep == c_partition) * expX   (DVE scalar_tensor_tensor)
  D     = sum_c expX    e_t = sum_c M     e'' = sum_c alpha_c * M
     -- all via PE "data-as-weights": lhsT = 128-col data block,
        rhs = Sel8 [128, 8] -> out[pos, group], full 128 partitions.
  lp = log e_t - log D (= log p_true);  a = exp(log e'' - log e_t) (= alpha_t)
  focal = a * (1 - p)^2 * (-lp);  loss = sum(focal)
Class counts: 16x 4x-mode is_equal masks + PE mask-reduce, AllReduce
across the 8 cores, alpha computed on-device -> weights of the e'' pass.
Host: sums per-core partial sums, divides by (numel + eps).
"""

import sys

sys.path.insert(0, "/opt/trn_rl_repo")

import numpy as np
import ml_dtypes

import bass_rust as _bass_rust
import concourse.bass as bass
import concourse.bacc as bacc
import concourse.tile as tile
from concourse import mybir
from concourse.bass_utils import run_bass_kernel_spmd
from concourse.hw_specs import get_activation_tables


class _Bacc(bacc.Bacc):
    def insert_act_table_loads(self):
        # Only Exp and Ln are used; keep them resolvable only via the
        # combined natural_log_exp set so a single ACT_TABLE_LOAD serves
        # the whole kernel (set ids must stay aligned with act_info.json,
        # so filter set contents instead of reordering).
        has_activation = any(
            isinstance(i, mybir.InstActivation)
            for b in self.main_func.blocks
            for i in b.instructions
        )
        if not has_activation:
            return
        AFT = mybir.ActivationFunctionType
        tables = []
        for name, fns in get_activation_tables(self.m.arch).items():
            if name != "natural_log_exp_and_others":
                fns = fns - {AFT.Exp, AFT.Ln}
            tables.append((name, fns))
        _bass_rust.insert_act_table_loads(self, tables)

# ---- problem constants (hardcoded; kernel.py must be self-contained) ----
N, C, H, W = 8, 16, 512, 512
POS = H * W          # positions per core = 262144
G = 8                # spatial groups -> partition = 16*g + c
FTOT = POS // G      # free columns in (g,c) layout = 32768
CHUNK = 2048         # sweep chunk columns
NCHUNK = FTOT // CHUNK          # 16
SUB = 512            # PSUM bank free width (fp32)
SC_COLS = 8192       # columns per super-chunk (-> [128, 512] position tiles)
NSC = FTOT // SC_COLS           # 4
CHUNKS_PER_SC = SC_COLS // CHUNK  # 4
TW = 128             # tpos free width per chunk-row  (POS/128 = 2048)

GAMMA = 2.0
SMOOTH = 1e-8
ALPHA_SMOOTH = 0.1

FP32 = mybir.dt.float32
BF16 = mybir.dt.bfloat16
AX = mybir.AxisListType
OP = mybir.AluOpType
AF = mybir.ActivationFunctionType


def build_nc(compile_graph=True, use_collective=True, use_late=True):
    nc = _Bacc("TRN2", target_bir_lowering=False, debug=False,
               num_devices=8)

    x_ext = nc.declare_dram_parameter("x", [128, FTOT], FP32, isOutput=False)
    tpos_ext = nc.declare_dram_parameter("tpos", [128, POS // 128], BF16,
                                         isOutput=False)
    sel8_ext = nc.declare_dram_parameter("sel8", [128, G], BF16, isOutput=False)
    b8_ext = nc.declare_dram_parameter("b8", [G, 128], BF16, isOutput=False)
    ones_ext = nc.declare_dram_parameter("ones128", [128, 1], FP32,
                                         isOutput=False)
    onesb_ext = nc.declare_dram_parameter("onesb", [128, 1], BF16,
                                          isOutput=False)
    ccol_ext = nc.declare_dram_parameter("ccol", [128, 1], FP32, isOutput=False)
    out_ext = nc.declare_dram_parameter("out", [128, NSC], FP32, isOutput=True)
    dbg_ext = nc.declare_dram_parameter("dbg", [4, C], FP32, isOutput=True)


    with tile.TileContext(nc) as tc:
        with (
            tc.tile_pool(name="singles", bufs=1) as singles,
            tc.tile_pool(name="xp", bufs=4) as xp,
            tc.tile_pool(name="mpool", bufs=NCHUNK) as mpool,
            tc.tile_pool(name="exp", bufs=3) as exp_pool,
            tc.tile_pool(name="pos", bufs=2) as pos_pool,
            tc.tile_pool(name="late", bufs=NSC) as late_pool,
            tc.tile_pool(name="tiny", bufs=2) as tiny,
            tc.tile_pool(name="psA", bufs=1, space="PSUM") as psA,
            tc.tile_pool(name="psE", bufs=1, space="PSUM") as psE,
            tc.tile_pool(name="psB", bufs=2, space="PSUM") as psB,
            tc.tile_pool(name="psT", bufs=2, space="PSUM") as psT,
            tc.tile_pool(name="dram", bufs=1, space="DRAM") as dram,
        ):
            # ---------------- constants / small inputs ----------------
            # DVE re-copies: hot-loop STT/LDW dependencies all ride the
            # single DVE semaphore (1 sync-wait slot per instruction).
            sel8_in = singles.tile([128, G], BF16)
            nc.sync.dma_start(out=sel8_in, in_=sel8_ext[:, :])
            sel8 = singles.tile([128, G], BF16)
            nc.vector.tensor_copy(out=sel8, in_=sel8_in)
            b8_in = singles.tile([G, 128], BF16)
            nc.sync.dma_start(out=b8_in, in_=b8_ext[:, :])
            b8 = singles.tile([G, 128], BF16)
            nc.vector.tensor_copy(out=b8, in_=b8_in)
            ones_in = singles.tile([128, 1], FP32)
            nc.sync.dma_start(out=ones_in, in_=ones_ext[:, :])
            ones128 = singles.tile([128, 1], FP32)
            nc.vector.tensor_copy(out=ones128, in_=ones_in)
            onesb_in = singles.tile([128, 1], BF16)
            nc.sync.dma_start(out=onesb_in, in_=onesb_ext[:, :])
            onesb = singles.tile([128, 1], BF16)
            nc.vector.tensor_copy(out=onesb, in_=onesb_in)
            ccol_in = singles.tile([128, 1], FP32)
            nc.sync.dma_start(out=ccol_in, in_=ccol_ext[:, :])
            ccol = singles.tile([128, 1], FP32)
            nc.vector.tensor_copy(out=ccol, in_=ccol_in)
            tpos = singles.tile([128, POS // 128], BF16)
            nc.gpsimd.dma_start(out=tpos, in_=tpos_ext[:, :])

            # ------------- histogram: 4x-mode masks + PE reduce -------------
            # cnt_colps[:, c] += sum over partitions of (tpos == c) blocks
            cnt_colps = psB.tile([128, C], FP32, tag="EPP")
            for c in range(C):
                scr = tiny.tile([128, POS // 128], BF16, tag="hscr")
                nc.vector.tensor_scalar(
                    out=scr, in0=tpos, scalar1=float(c), scalar2=None,
                    op0=OP.is_equal,
                )
                nblk = (POS // 128) // 128
                for b in range(nblk):
                    nc.tensor.matmul(
                        cnt_colps[:, c:c + 1],
                        lhsT=scr[:, 128 * b:128 * (b + 1)], rhs=onesb,
                        start=(b == 0), stop=(b == nblk - 1),
                    )
            cnt_col = singles.tile([128, C], FP32)
            nc.vector.tensor_copy(out=cnt_col, in_=cnt_colps)
            cnt_ps = psA.tile([1, C], FP32, tag="D")
            nc.tensor.matmul(cnt_ps, lhsT=ones128, rhs=cnt_col, start=True,
                             stop=True)
            cnt_sb = singles.tile([1, C], FP32)
            nc.vector.tensor_copy(out=cnt_sb, in_=cnt_ps)

            cnt_g = singles.tile([1, C], FP32)
            if use_collective:
                cc_in = dram.tile([1, C], FP32)
                cc_out = dram.tile([1, C], FP32)
                nc.gpsimd.dma_start(out=cc_in[:], in_=cnt_sb)
                nc.gpsimd.collective_compute(
                    "AllReduce", OP.add,
                    replica_groups=[list(range(8))],
                    ins=[cc_in.opt()], outs=[cc_out.opt()],
                )
                nc.gpsimd.dma_start(out=cnt_g, in_=cc_out[:])
            else:
                nc.vector.tensor_scalar_mul(cnt_g, cnt_sb, 8.0)

            # ---------------- alpha from global counts ----------------
            wv = singles.tile([1, C], FP32)
            nc.vector.tensor_scalar(
                out=wv, in0=cnt_g, scalar1=1.0 / float(N * POS),
                scalar2=ALPHA_SMOOTH, op0=OP.mult, op1=OP.add,
            )
            nc.vector.reciprocal(out=wv, in_=wv)
            pres = singles.tile([1, C], FP32)
            nc.vector.tensor_scalar(
                out=pres, in0=cnt_g, scalar1=0.0, scalar2=None, op0=OP.is_gt,
            )
            wp = singles.tile([1, C], FP32)
            nc.vector.tensor_mul(wp, wv, pres)
            wsum = singles.tile([1, 1], FP32)
            nc.vector.tensor_reduce(out=wsum, in_=wp, axis=AX.X, op=OP.add)
            nc.vector.reciprocal(out=wsum, in_=wsum)
            alpha = singles.tile([1, C], FP32)
            nc.vector.tensor_scalar(
                out=alpha, in0=wp, scalar1=wsum, scalar2=None, op0=OP.mult,
            )
            omp = singles.tile([1, C], FP32)
            nc.vector.tensor_scalar(
                out=omp, in0=pres, scalar1=-1.0, scalar2=1.0,
                op0=OP.mult, op1=OP.add,
            )
            nc.vector.tensor_add(alpha, alpha, omp)

            nc.gpsimd.dma_start(out=dbg_ext[0:1, :], in_=cnt_g)
            nc.gpsimd.dma_start(out=dbg_ext[1:2, :], in_=alpha)

            # alpha -> [128,1] column (alpha_col[p] = alpha[p % 16])
            al_dram = dram.tile([1, C], FP32)
            nc.gpsimd.dma_start(out=al_dram[:], in_=alpha)
            alpha_in = singles.tile([128, 1], FP32)
            al_bcast = bass.AP(
                tensor=al_dram.tensor,
                offset=al_dram.offset,
                ap=[[0, G], [1, C], [1, 1]],
            )
            nc.gpsimd.dma_start(out=alpha_in, in_=al_bcast)
            alpha_col = singles.tile([128, 1], FP32)
            nc.vector.tensor_copy(out=alpha_col, in_=alpha_in)
            sel8a = singles.tile([128, G], BF16)
            nc.vector.tensor_scalar(
                out=sel8a, in0=sel8, scalar1=alpha_col, scalar2=None,
                op0=OP.mult,
            )

            # ---------------- main sweep ----------------
            loss_col = singles.tile([128, NSC], FP32)
            m_tiles = {}
            d_tiles = {}
            e_tiles = {}
            le_tiles = {}
            f1_tiles = {}

            for k in range(NCHUNK):
                col0 = k * CHUNK
                cols = slice(col0, col0 + CHUNK)

                x_t = xp.tile([128, CHUNK], FP32, tag="x")
                # alternate DMA queue groups: HWDGE (0-7) / SWDGE (8-15)
                dma_eng = nc.sync if k % 2 == 0 else nc.gpsimd
                dma_eng.dma_start(out=x_t, in_=x_ext[:, cols])

                ex = exp_pool.tile([128, CHUNK], BF16, tag="ex")
                nc.scalar.activation(out=ex, in_=x_t, func=AF.Exp)

                s, j0 = divmod(k, CHUNKS_PER_SC)
                if j0 == 0:
                    d_tile = psA.tile([128, SUB], FP32, tag="D")
                    e_tile = psE.tile([128, SUB], FP32, tag="E")
                    d_tiles[s] = d_tile
                    e_tiles[s] = e_tile

                # T_rep via PE broadcast: trep_ps[16g+c, f] = tpos[8k+g, f].
                # matmul rhs must sit at partition 0 -> stage the 8 rows.
                m_t = mpool.tile([128, CHUNK], BF16, tag="m")
                m_tiles[k] = m_t
                tstage = tiny.tile([G, CHUNK], BF16, tag="tstage")
                nc.sync.dma_start(out=tstage, in_=tpos[G * k:G * k + G, :])
                for h in range(2):
                    hw = CHUNK // 2
                    hc = slice(col0 + h * hw, col0 + (h + 1) * hw)
                    hl = slice(h * hw, (h + 1) * hw)
                    trep_ps = psT.tile([128, hw], FP32, tag="trep")
                    for q in range(hw // SUB):
                        nc.tensor.matmul(
                            trep_ps[:, q * SUB:(q + 1) * SUB],
                            lhsT=b8,
                            rhs=tstage[:, h * hw + q * SUB:
                                       h * hw + (q + 1) * SUB],
                            start=True, stop=True,
                        )
                    nc.vector.scalar_tensor_tensor(
                        out=m_t[:, hl], in0=trep_ps, scalar=ccol,
                        in1=ex[:, hl], op0=OP.is_equal, op1=OP.mult,
                    )

                # D / e_t via data-as-weights matmuls
                for j in range(CHUNK // 128):
                    bb = j0 * (CHUNK // 128) + j  # block within super-chunk
                    blk = slice(col0 + j * 128, col0 + (j + 1) * 128)
                    nc.tensor.matmul(
                        d_tiles[s][:, 8 * bb:8 * bb + 8],
                        lhsT=ex[:, j * 128:(j + 1) * 128], rhs=sel8,
                        start=True, stop=True,
                    )
                    nc.tensor.matmul(
                        e_tiles[s][:, 8 * bb:8 * bb + 8],
                        lhsT=m_t[:, j * 128:(j + 1) * 128], rhs=sel8,
                        start=True, stop=True,
                    )

                if j0 == CHUNKS_PER_SC - 1:
                    # early epilogue for super-chunk s  (ACT: Exp/Ln/Square
                    # + affine Copy only -> single table set)
                    lD = pos_pool.tile([128, SUB], FP32, tag="lD")
                    nc.scalar.activation(out=lD, in_=d_tiles[s], func=AF.Ln)
                    le = late_pool.tile([128, SUB], FP32, tag="le")
                    nc.scalar.activation(out=le, in_=e_tiles[s], func=AF.Ln)
                    le_tiles[s] = le
                    lp = pos_pool.tile([128, SUB], FP32, tag="lp")
                    nc.vector.tensor_sub(lp, le, lD)
                    p = pos_pool.tile([128, SUB], FP32, tag="p")
                    nc.scalar.activation(out=p, in_=lp, func=AF.Exp)
                    u_t = pos_pool.tile([128, SUB], FP32, tag="u")
                    nc.vector.tensor_scalar(
                        out=u_t, in0=p, scalar1=-1.0, scalar2=1.0,
                        op0=OP.mult, op1=OP.add)
                    w_t = pos_pool.tile([128, SUB], FP32, tag="w")
                    nc.vector.tensor_mul(w_t, u_t, u_t)
                    f1 = late_pool.tile([128, SUB], FP32, tag="f1")
                    nc.vector.tensor_mul(f1, w_t, lp)
                    f1_tiles[s] = f1

            # ---------------- alpha-weighted pass + late epilogue ----------
            for s in range(NSC if use_late else 0):
                epp = psB.tile([128, SUB], FP32, tag="EPP")
                for bb in range(SC_COLS // 128):
                    col0 = s * SC_COLS + bb * 128
                    kk, off = divmod(col0, CHUNK)
                    nc.tensor.matmul(
                        epp[:, 8 * bb:8 * bb + 8],
                        lhsT=m_tiles[kk][:, off:off + 128], rhs=sel8a,
                        start=True, stop=True,
                    )
                lepp = pos_pool.tile([128, SUB], FP32, tag="lepp")
                nc.scalar.activation(out=lepp, in_=epp, func=AF.Ln)
                la = pos_pool.tile([128, SUB], FP32, tag="la")
                nc.vector.tensor_sub(la, lepp, le_tiles[s])
                a_t = pos_pool.tile([128, SUB], FP32, tag="a")
                nc.scalar.activation(out=a_t, in_=la, func=AF.Exp)
                f2 = pos_pool.tile([128, SUB], FP32, tag="f2")
                nc.vector.tensor_mul(f2, f1_tiles[s], a_t)
                nc.vector.tensor_reduce(
                    out=loss_col[:, s:s + 1], in_=f2, axis=AX.X, op=OP.add)

            if not use_late:
                for s in range(NSC):
                    nc.vector.tensor_reduce(
                        out=loss_col[:, s:s + 1], in_=f1_tiles[s],
                        axis=AX.X, op=OP.add)

            nc.sync.dma_start(out=out_ext[:, :], in_=loss_col)

    if compile_graph:
        nc.compile()
    return nc


_CACHED = {}


def _get_nc():
    if "nc" not in _CACHED:
        _CACHED["nc"] = build_nc()
    return _CACHED["nc"]


def make_in_maps(logits, target):
    logits = np.ascontiguousarray(np.asarray(logits, dtype=np.float32))
    target = np.asarray(target)

    sel8 = np.zeros((128, G), dtype=ml_dtypes.bfloat16)
    for p in range(128):
        sel8[p, p // C] = 1.0
    b8 = np.zeros((G, 128), dtype=ml_dtypes.bfloat16)
    for m in range(128):
        b8[m // C, m] = 1.0
    ones128 = np.ones((128, 1), dtype=np.float32)
    onesb = np.ones((128, 1), dtype=ml_dtypes.bfloat16)
    ccol = (np.arange(128, dtype=np.float32) % C).reshape(128, 1)

    in_maps = []
    for n in range(N):
        t_flat = target[n].reshape(-1).astype(np.float32)
        # logits in (g,c)-layout: row 16g+c = logits[c, g*FTOT : (g+1)*FTOT]
        x128 = np.ascontiguousarray(np.transpose(
            logits[n].reshape(C, G, FTOT), (1, 0, 2)).reshape(128, FTOT))
        # tpos layout: partition (8k + g) = t[g*FTOT + k*2048 : +2048]
        tpos = np.transpose(
            t_flat.reshape(G, 16, 2048), (1, 0, 2)).reshape(128, 2048)
        in_maps.append({
            "x": x128,
            "tpos": np.ascontiguousarray(tpos).astype(ml_dtypes.bfloat16),
            "sel8": sel8,
            "b8": b8,
            "ones128": ones128,
            "onesb": onesb,
            "ccol": ccol,
        })
    return in_maps


def combine(results):
    total = 0.0
    for r in results:
        total += np.asarray(r["out"], dtype=np.float64).sum()
    loss = -total / (float(N * POS) + SMOOTH)
    return np.float32(loss)


def kernel(logits, target, trace=False, **run_kwargs):
    nc = _get_nc()
    in_maps = make_in_maps(logits, target)
    res = run_bass_kernel_spmd(nc, in_maps, core_ids=list(range(8)),
                               trace=trace, **run_kwargs)
    out = combine(res.results)
    if trace:
        kernel.last_result = res
    return out
